# revision 30
# baseline (speedup 1.0000x reference)
"""Causal multi-head attention (B=2, S=2048, D=1024, 16 heads of 64) on 8 TRN2
NeuronCores.

Sharding: core c -> batch b = c//4, head-group g = c%4 (4 heads = 256 model
dims per core).  Wq/Wk/Wv column-parallel, Wo row-parallel; the 4 partial
outputs per batch are summed on the host (no collectives).

Per-core data flow (matmul compute in bf16, fp32 PSUM accumulation).
Every matmul is a full 128x128-mode op (no PE tiling-mode switches, weight
loads pipeline in the background buffer):
  V  = x @ Wv_g^T + bv     [2048, 256]  natural layout, ones-augmented col
  QZ = (Wq_g/8) @ x^T      kept as 4 zero-padded tiles: qz[2p+h] has head
       (2p+h)'s 64 dims in their native partition rows, 0 elsewhere, so the
       score matmul can contract over K=128 (junk K rows hit zero Q rows;
       matmul time depends only on N, so the padding is free)
  KT = Wk_g @ x^T          [256, 2048]
  attention per head pair:
    ST_h[sk,sq] = KT_tile^T-contract against qz[2p+h]   [128, 512] PSUM
    P = exp(ST) masked      -- ACT exp on the causal-needed region only
    AV: K=128 matmuls, out[65,512] (ones-augmented V row 64 = l),
        software-pipelined 2 ki behind scores
    normalize: recip on l row [1,512], gpsimd partition-broadcast, DVE mul
  out_partial = preoutT.T @ Wo_g^T   [2048, 1024]
Host: out[b] = sum of the 4 head-group partials + bo.

Engine split: PE matmuls; ACT exp + Q/K bias evac (activation Identity w/
bias) + half the Wo evac; DVE V evac, masks, AV evac, rest of Wo evac;
Pool the partition broadcasts.  DMA: V-proj inputs first in small striped
pieces so the PE starts early; output tiles striped to kill the DMA tail.
"""

import os

import numpy as np
import ml_dtypes

B, S, D = 2, 2048, 1024
HD = 64
NH = D // HD
N_CORES = 8
GROUPS = 4          # head-groups (tensor-parallel)
JG = D // GROUPS    # local dims per core = 256
NHL = JG // HD      # local heads = 4
KCH = D // 128      # contraction chunks for projections = 8
NKT = S // 128      # sk tiles = 16
NJB = S // 512      # query blocks of 512 = 4

BF16 = ml_dtypes.bfloat16

_cached = {}


def _build():
    import concourse.bacc as bacc
    import concourse.tile as tile
    import concourse.mybir as mybir

    f32 = mybir.dt.float32
    bf16 = mybir.dt.bfloat16
    Exp = mybir.ActivationFunctionType.Exp

    nc = bacc.Bacc("TRN2", target_bir_lowering=False, debug=False,
                   num_devices=N_CORES)

    # x in si-major tiling: xT[p, si, k, c] = x[128*si + c, 128*k + p]
    xT = nc.dram_tensor("xT", [128, NKT, KCH, 128], bf16, kind="ExternalInput").ap()
    wqT = nc.dram_tensor("wqT", [128, KCH, JG], bf16, kind="ExternalInput").ap()
    wkT = nc.dram_tensor("wkT", [128, KCH, JG], bf16, kind="ExternalInput").ap()
    wvT = nc.dram_tensor("wvT", [128, KCH, JG], bf16, kind="ExternalInput").ap()
    woT = nc.dram_tensor("woT", [128, 2, D], bf16, kind="ExternalInput").ap()
    bqc = nc.dram_tensor("bqc", [JG, 1], f32, kind="ExternalInput").ap()
    bkc = nc.dram_tensor("bkc", [JG, 1], f32, kind="ExternalInput").ap()
    bvb = nc.dram_tensor("bvb", [128, JG], f32, kind="ExternalInput").ap()
    maskT = nc.dram_tensor("maskT", [128, 128], bf16, kind="ExternalInput").ap()
    out = nc.dram_tensor("out", [S, D], bf16, kind="ExternalOutput").ap()

    with tile.TileContext(nc) as tc:
        with (
            tc.tile_pool(name="const", bufs=1) as cpool,
            tc.tile_pool(name="pbig", bufs=2) as p_pool,
            tc.tile_pool(name="small", bufs=4) as small_pool,
            tc.tile_pool(name="tailp", bufs=1) as tail_pool,
            tc.tile_pool(name="outp", bufs=8) as out_pool,
            tc.tile_pool(name="mm_ps", bufs=2, space="PSUM") as mm_ps,   # 2 banks
            tc.tile_pool(name="st_ps", bufs=4, space="PSUM") as st_ps,   # 4 banks
            tc.tile_pool(name="av_ps", bufs=2, space="PSUM") as av_ps,   # 2 banks
        ):
            # ---- SBUF tiles ----
            wv_sb = cpool.tile([128, KCH, JG], bf16)
            xt_all = cpool.tile([128, NKT, KCH, 128], bf16)
            bvb_sb = cpool.tile([128, JG], f32)
            mask_sb = cpool.tile([128, 128], bf16)
            wq_sb = cpool.tile([128, KCH, JG], bf16)
            bq_sb = cpool.tile([128, 2], f32)
            wk_sb = cpool.tile([128, KCH, JG], bf16)
            bk_sb = cpool.tile([128, 2], f32)
            wo_sb = cpool.tile([128, 2, D], bf16)

            # ---- DMA: consumption order, three doorbell rings ----
            def xt_piece(si, q, nq, e):
                kw = KCH // nq
                e.dma_start(xt_all[:, si, kw * q:kw * q + kw, :],
                            xT[:, si, kw * q:kw * q + kw, :])

            A, P_, Y = nc.scalar, nc.gpsimd, nc.sync
            # ACT rings the weights (it is otherwise idle until attention
            # starts), SP rings x pieces in consumption order, GpSimd the
            # late wo.
            for k in range(4):
                A.dma_start(wv_sb[:, 2 * k:2 * k + 2, :],
                            wvT[:, 2 * k:2 * k + 2, :])
                xt_piece(0, k, 4, Y)
            Y.dma_start(bvb_sb[:], bvb[:])
            for k in range(4):
                A.dma_start(wq_sb[:, 2 * k:2 * k + 2, :],
                            wqT[:, 2 * k:2 * k + 2, :])
                xt_piece(1, k, 4, Y)
            for k in range(4):
                A.dma_start(wk_sb[:, 2 * k:2 * k + 2, :],
                            wkT[:, 2 * k:2 * k + 2, :])
            for k in range(2):
                xt_piece(2, k, 2, Y)
                xt_piece(3, k, 2, Y)
            P_.dma_start(bq_sb[:], bqc.rearrange("(t p) o -> p (t o)", p=128))
            P_.dma_start(bk_sb[:], bkc.rearrange("(t p) o -> p (t o)", p=128))
            P_.dma_start(mask_sb[:], maskT[:])
            for si in range(4, NKT):
                xt_piece(si, 0, 2, Y)
                xt_piece(si, 1, 2, Y)
            for q in range(4):
                P_.dma_start(wo_sb[:, :, 256 * q:256 * q + 256],
                             woT[:, :, 256 * q:256 * q + 256])

            # dummy matmuls on a zeroed scratch tile: warm the PE/HAM while
            # the first input DMA pieces are still in flight (emitted before
            # every other DVE op so the scratch zeroing runs immediately)
            scr = cpool.tile([128, 256], bf16, name="scr")
            nc.vector.memset(scr[:], 0.0)
            _nwarm = [0]

            def pe_warm(n_mm, nn=256):
                for _ in range(n_mm):
                    w_ = _nwarm[0]
                    _nwarm[0] += 1
                    wps = st_ps.tile([128, 512], f32, tag="st",
                                     name=f"wps{w_}")
                    nc.tensor.matmul(wps[:, 0:nn], lhsT=scr[:, 0:128],
                                     rhs=scr[:, 0:nn], start=True, stop=True)

            pe_warm(4)

            # qz[2p+h]: head (2p+h)'s Q dims in rows 64h..64h+63, 0 elsewhere
            qz = [cpool.tile([128, S], bf16, name=f"qz{i}") for i in range(4)]
            kt = [cpool.tile([128, S], bf16, name=f"kt{t}") for t in range(2)]
            for i in range(4):
                hh = i % 2
                nc.vector.memset(qz[i][64 * (1 - hh):64 * (1 - hh) + 64, :], 0.0)
            v_all = cpool.tile([128, NKT, NHL * 65], bf16)
            nc.vector.memset(
                v_all.rearrange("p k (h c) -> p k h c", c=65)[:, :, :, 64:65], 1.0)
            po = [cpool.tile([128, S], bf16, name=f"po{t}") for t in range(2)]
            warm = small_pool.tile([1, 4], f32, tag="rinv")
            nc.vector.memset(warm[:], 0.0)
            nc.scalar.activation(warm[:], warm[:], Exp)
            ones_b = cpool.tile([1, 64], bf16, name="ones_b")
            nc.vector.memset(ones_b[:], 1.0)

            # ---- phase helpers ----
            def proj_v(si):
                ps = mm_ps.tile([128, 512], f32, tag="mm", name=f"psv{si}")
                for k in range(KCH):
                    nc.tensor.matmul(
                        ps[:, 0:256],
                        lhsT=xt_all[:, si, k, :],
                        rhs=wv_sb[:, k, :],
                        start=(k == 0), stop=(k == KCH - 1))
                nc.vector.tensor_add(
                    v_all[:, si, :].rearrange(
                        "p (h c) -> p h c", c=65)[:, :, 0:64],
                    ps[:, 0:256].rearrange("p (h c) -> p h c", c=64),
                    bvb_sb.rearrange("p (h c) -> p h c", c=64))

            def proj_q(t, n):
                ps = mm_ps.tile([128, 512], f32, tag="mm", name=f"psq{t}{n}")
                for k in range(KCH):
                    nc.tensor.matmul(
                        ps[:],
                        lhsT=wq_sb[:, k, 128 * t:128 * t + 128],
                        rhs=xt_all[:, 4 * n:4 * n + 4, k, :],
                        start=(k == 0), stop=(k == KCH - 1))
                # split-evac into the two zero-padded head tiles
                for hh in range(2):
                    r = slice(64 * hh, 64 * hh + 64)
                    nc.scalar.add(qz[2 * t + hh][r, 512 * n:512 * n + 512],
                                  ps[r, :], bq_sb[r, t:t + 1])

            def proj_k(t, n):
                ps = mm_ps.tile([128, 512], f32, tag="mm", name=f"psk{t}{n}")
                for k in range(KCH):
                    nc.tensor.matmul(
                        ps[:],
                        lhsT=wk_sb[:, k, 128 * t:128 * t + 128],
                        rhs=xt_all[:, 4 * n:4 * n + 4, k, :],
                        start=(k == 0), stop=(k == KCH - 1))
                nc.scalar.add(
                    kt[t][:, 512 * n:512 * n + 512], ps[:], bk_sb[:, t:t + 1])

            def attn_scores(pair, j, ki, pt_all):
                d = max(0, 128 * ki - 512 * j)
                sts = [st_ps.tile([128, 512], f32, tag="st",
                                  name=f"st{pair}_{j}_{ki}_{hh}")
                       for hh in range(2)]
                for hh in range(2):
                    nc.tensor.matmul(
                        sts[hh][:, d:512],
                        lhsT=kt[pair][:, 128 * ki:128 * ki + 128],
                        rhs=qz[2 * pair + hh][:, 512 * j + d:512 * j + 512],
                        start=True, stop=True)
                for hh in range(2):
                    nc.scalar.activation(
                        pt_all[:, ki, 512 * hh + d:512 * hh + 512],
                        sts[hh][:, d:512], Exp)
                if ki >= 4 * j:
                    for hh in range(2):
                        nc.vector.tensor_mul(
                            pt_all[:, ki, 512 * hh + d:512 * hh + d + 128],
                            pt_all[:, ki, 512 * hh + d:512 * hh + d + 128],
                            mask_sb[:])

            def attn_av_mm(pair, j, ki, nk, pt_all, pos):
                d = max(0, 128 * ki - 512 * j)
                for hh in range(2):
                    h = 2 * pair + hh
                    nc.tensor.matmul(
                        pos[hh][0:65, d:512],
                        lhsT=v_all[:, ki, 65 * h:65 * h + 65],
                        rhs=pt_all[:, ki, 512 * hh + d:512 * hh + 512],
                        start=(ki == 0), stop=(ki == nk - 1))

            def attn_evac(pair, j, pos, tail=None):
                # tail: interleave the final output tiles at 256-col chunks
                # so the last Wo matmuls start as soon as their po columns
                # are normalized instead of after the whole 512-col chain.
                # On the tail block the 1/l broadcast runs as a K=1 matmul
                # (ones x rinv) — the PE is idle there and the gpsimd
                # partition-broadcast's 1.2us latency would gate the tail.
                ss = []
                if tail:
                    # latency-lean variant: l rows straight out of PSUM,
                    # one combined reciprocal+cast for both heads, then two
                    # K=1 broadcast matmuls (the PE is idle here; scratch
                    # matmuls keep it warm through the chain latency)
                    lrow2 = tail_pool.tile([1, 1024], f32, tag="lrow2")
                    for hh in range(2):
                        nc.vector.tensor_copy(lrow2[:, 512 * hh:512 * hh + 512],
                                              pos[hh][64:65, :])
                    rinv2 = tail_pool.tile([1, 1024], f32, tag="rinv2")
                    nc.vector.reciprocal_approx_fast(rinv2[:], lrow2[:])
                    rinv_b = tail_pool.tile([1, 1024], bf16, tag="rinvb2")
                    nc.vector.tensor_copy(rinv_b[:], rinv2[:])
                    pe_warm(6)
                    rbs = []
                    for hh in range(2):
                        rb = st_ps.tile([64, 512], f32, tag="st",
                                        name=f"rbps{hh}")
                        nc.tensor.matmul(rb[:], lhsT=ones_b[:, 0:64],
                                         rhs=rinv_b[:, 512 * hh:512 * hh + 512],
                                         start=True, stop=True)
                        rbs.append(rb)
                    for hh in range(2):
                        ssum = small_pool.tile([65, 512], f32, tag="ssum")
                        nc.vector.tensor_copy(ssum[:], pos[hh][:])
                        ss.append((ssum, rbs[hh]))
                else:
                    for hh in range(2):
                        ssum = small_pool.tile([65, 512], f32, tag="ssum")
                        nc.vector.tensor_copy(ssum[:], pos[hh][:])
                        lrow = small_pool.tile([1, 512], f32, tag="lrow")
                        nc.vector.tensor_copy(lrow[:], ssum[64:65, :])
                        rinv = small_pool.tile([1, 512], f32, tag="rinv")
                        nc.vector.reciprocal_approx_fast(rinv[:], lrow[:])
                        rb = small_pool.tile([64, 512], f32, tag="rb")
                        nc.gpsimd.partition_broadcast(rb[:], rinv[:])
                        ss.append((ssum, rb))
                for c in range(2):
                    cs = slice(256 * c, 256 * c + 256)
                    for hh in range(2):
                        ssum, rb = ss[hh]
                        nc.vector.tensor_mul(
                            po[pair][64 * hh:64 * hh + 64,
                                     512 * j + 256 * c:512 * j + 256 * c + 256],
                            ssum[0:64, cs], rb[:, cs])
                    if tail:
                        for m in (4 * j + 2 * c, 4 * j + 2 * c + 1):
                            for n_ in range(2):
                                wo_half(m, n_)

            LAG = 2

            def attn_block(pair, j, fills=()):
                nk = 4 * (j + 1)
                fills = list(fills)
                if os.environ.get("K_NO_FILLS"):
                    for f in fills:
                        f()
                    fills = []
                pt_all = p_pool.tile([128, NKT, 1024], bf16, tag="p",
                                     name=f"pt{pair}_{j}")
                pos = [av_ps.tile([65, 512], f32, tag="po",
                                  name=f"pos{pair}_{j}_{hh}")
                       for hh in range(2)]
                for ki in range(nk + LAG):
                    if ki < nk:
                        attn_scores(pair, j, ki, pt_all)
                    if ki >= LAG:
                        attn_av_mm(pair, j, ki - LAG, nk, pt_all, pos)
                    if fills and ki % 2 == 1:
                        fills.pop(0)()
                attn_evac(pair, j, pos, tail=((pair, j) == (1, 3)))
                for f in fills:
                    f()

            def wo_half(m, n, only_t=None):
                # only_t: 0 -> first contraction half (start, no stop);
                #         1 -> finish an open group; None -> both
                if only_t == 1:
                    ps = _wo_ps.pop((m, n))
                else:
                    ps = mm_ps.tile([128, 512], f32, tag="mm",
                                    name=f"pswo{m}_{n}")
                    if only_t == 0:
                        _wo_ps[(m, n)] = ps
                for t in ((0, 1) if only_t is None else (only_t,)):
                    nc.tensor.matmul(
                        ps[:],
                        lhsT=po[t][:, 128 * m:128 * m + 128],
                        rhs=wo_sb[:, t, 512 * n:512 * n + 512],
                        start=(t == 0), stop=(t == 1))
                if only_t == 0:
                    return
                ob = out_pool.tile([128, 512], bf16, tag="ob")
                # tail tiles always evacuate on ACT: the DVE is busy with
                # the final normalize muls there and exp is already done
                if m < 12 and (m + n) % 2 == 0:
                    nc.vector.tensor_copy(ob[:], ps[:])
                else:
                    nc.scalar.copy(ob[:], ps[:])
                # stripe harder toward the tail so the last tiles drain
                # fast; n=1 halves ring on the ACT DGE so the two doorbell
                # queues share the issue load
                npc = 4 if m >= 12 else (2 if m >= 10 else 1)
                w = 128 // npc
                ring = nc.sync if n == 0 else nc.scalar
                for q in range(npc):
                    ring.dma_start(
                        out[128 * m + w * q:128 * m + w * q + w,
                            512 * n:512 * n + 512],
                        ob[w * q:w * q + w, :])

            _wo_ps = {}

            # ---- schedule ----
            proj_v(0), proj_v(1), proj_v(2), proj_v(3)
            proj_q(0, 0), proj_k(0, 0)
            attn_block(0, 0, fills=[lambda: proj_v(4), lambda: proj_v(5)])
            proj_q(1, 0), proj_k(1, 0)
            attn_block(1, 0, fills=[lambda: proj_v(6), lambda: proj_v(7)])
            proj_q(0, 1), proj_k(0, 1)
            proj_q(1, 1), proj_k(1, 1)
            attn_block(0, 1, fills=[lambda: proj_v(8), lambda: proj_v(9),
                                    lambda: proj_v(10), lambda: proj_v(11)])
            attn_block(1, 1, fills=[lambda: proj_v(12), lambda: proj_v(13),
                                    lambda: proj_v(14), lambda: proj_v(15)])
            proj_q(0, 2), proj_k(0, 2)
            proj_q(1, 2), proj_k(1, 2)
            attn_block(0, 2, fills=[
                lambda: wo_half(0, 0), lambda: wo_half(0, 1),
                lambda: wo_half(1, 0), lambda: wo_half(1, 1),
                lambda: wo_half(2, 0), lambda: wo_half(2, 1)])
            attn_block(1, 2, fills=[
                lambda: wo_half(3, 0), lambda: wo_half(3, 1),
                lambda: (proj_q(0, 3), proj_k(0, 3)),
                lambda: (proj_q(1, 3), proj_k(1, 3))])
            attn_block(0, 3, fills=[
                lambda: wo_half(4, 0), lambda: wo_half(4, 1),
                lambda: wo_half(5, 0), lambda: wo_half(5, 1),
                lambda: wo_half(6, 0), lambda: wo_half(6, 1),
                lambda: wo_half(7, 0), lambda: wo_half(7, 1)])
            # m12-15 are emitted inside attn_block(1,3)'s chunked evac tail
            attn_block(1, 3, fills=[
                lambda: wo_half(8, 0), lambda: wo_half(8, 1),
                lambda: wo_half(9, 0), lambda: wo_half(9, 1),
                lambda: wo_half(10, 0), lambda: wo_half(10, 1),
                lambda: wo_half(11, 0), lambda: wo_half(11, 1),
            ])

    nc.compile()
    return nc


def _get_nc():
    if "nc" not in _cached:
        _cached["nc"] = _build()
    return _cached["nc"]


def _make_in_maps(x, Wq, bq, Wk, bk, Wv, bv, Wo):
    sc = 1.0 / np.sqrt(HD)
    tri = np.arange(128)
    mask = np.where(tri[:, None] <= tri[None, :], 1.0, 0.0).astype(BF16)
    in_maps = []
    for c in range(N_CORES):
        b, g = divmod(c, GROUPS)
        sl = slice(JG * g, JG * (g + 1))

        def tile_k(a):  # [D, M] -> [128, D//128, M] contiguous
            return np.ascontiguousarray(
                a.reshape(a.shape[0] // 128, 128, a.shape[1]).transpose(1, 0, 2))

        # xT[p, si, k, c] = x[b][128*si + c, 128*k + p]
        xt2 = np.ascontiguousarray(
            x[b].astype(BF16).reshape(NKT, 128, KCH, 128).transpose(3, 0, 2, 1))

        in_maps.append({
            "xT": xt2,
            "wqT": tile_k((Wq[sl] * sc).T.astype(BF16)),
            "wkT": tile_k(Wk[sl].T.astype(BF16)),
            "wvT": tile_k(Wv[sl].T.astype(BF16)),
            "woT": tile_k(Wo[:, sl].T.astype(BF16)),
            "bqc": (bq[sl] * sc).astype(np.float32).reshape(JG, 1),
            "bkc": bk[sl].astype(np.float32).reshape(JG, 1),
            "bvb": np.broadcast_to(bv[sl].astype(np.float32), (128, JG)).copy(),
            "maskT": mask,
        })
    return in_maps


def kernel(x, Wq, bq, Wk, bk, Wv, bv, Wo, bo, _return_results=False):
    from concourse.bass_utils import run_bass_kernel_spmd

    nc = _get_nc()
    in_maps = _make_in_maps(np.asarray(x, np.float32), np.asarray(Wq, np.float32),
                            np.asarray(bq, np.float32), np.asarray(Wk, np.float32),
                            np.asarray(bk, np.float32), np.asarray(Wv, np.float32),
                            np.asarray(bv, np.float32), np.asarray(Wo, np.float32))
    res = run_bass_kernel_spmd(nc, in_maps, core_ids=list(range(N_CORES)))
    full = np.empty((B, S, D), np.float32)
    for b in range(B):
        acc = res.results[4 * b]["out"].astype(np.float32).copy()
        for g in range(1, GROUPS):
            acc += res.results[4 * b + g]["out"]
        full[b] = acc + np.asarray(bo, np.float32)[None, :]
    if _return_results:
        return full, res
    return full
